# revision 17
# baseline (speedup 1.0000x reference)
"""Self-contained GCN encoder kernel for 8 TRN2 NeuronCores (Bass/Tile).

kernel(**inputs) takes the FULL unsharded inputs (as from setup_inputs())
and returns the FULL [50000, 64] float32 output.

Strategy: dst-node tiles of 128 are LPT-balanced across 8 cores; the
NEFF is specialized per run to the edge structure. Embedding lookups are
done host-side and shipped channel-major; stage 1 is pure matmul with
per-tile feature DMAs. Each conv's node table is split into THREE
sub-tables ([8,20,21] slots x 128 x 8 cores) so the first AllGather fires
after only 8 stage-1 tiles; the remaining stage-1 tiles and the other
AllGather triggers are deferred into the conv-1 op stream so neither the
Q7 queue nor the PE queue serializes behind them. Per (8-dst-tile group,
sub-table) gather ops use dma_gather (the Q7 descriptor-generation rate
of ~8ns/row is the kernel's roofline; the Pool engine runs >98% busy
between first and last gather). Segment-reduction is on the TensorEngine
via is_equal selection matrices into PSUM (tile-major pair order keeps
<=2 banks live per op) with per-phase SBUF spills; the self-loop term is
an identity matmul of SBUF-resident h tiles and the bias is a rank-1
((1/dinv) x b) matmul, so the epilogue is one DVE add + one fused
Relu-scale activation on the Scalar engine (keeps the DVE queue free of
head-of-line stalls). The six AllGathers are staggered via per-table
op-stream delays so neither conv ever waits on a collective.
"""
import numpy as np
import ml_dtypes
from concourse import bass, bacc, mybir, tile
from concourse.bass_utils import run_bass_kernel_spmd
from concourse.masks import make_identity

P = 128
CORES = 8
N = 50000
NTILES = 392
NPAD = NTILES * P     # 50176
TPC = NTILES // CORES  # 49
NLOC = TPC * P        # 6272
SLOTS = [8, 20, 21]   # slots per sub-table (sum = TPC)
SLOT0 = [0, 8, 28]    # first slot of each sub-table
NTAB = 3
ROWS_T = [s * P for s in SLOTS]          # per-core rows per table
TROWS = [CORES * r for r in ROWS_T]      # total rows per table
C1 = 128
C2 = 64
IN_CH = 136
PAD_DSTL = 30000.0
G_MERGE = 8
NGROUPS = (TPC + G_MERGE - 1) // G_MERGE
DELAYS = [0, 3, 5]    # stagger (in rounds) of the three table streams


def wrap_idx(arr):
    return arr.reshape(-1, 16).T


def rup(x, m):
    return int((x + m - 1) // m * m)


def tab_of_slot(k):
    if k < SLOT0[1]:
        return 0
    if k < SLOT0[2]:
        return 1
    return 2


def prep(x, edge_index, emb_a, emb_b, W1, b1, W2, b2):
    src, dst = np.asarray(edge_index[0]), np.asarray(edge_index[1])
    deg = np.bincount(dst, minlength=N).astype(np.float32) + 1.0
    dinv = (1.0 / np.sqrt(deg)).astype(np.float32)

    # ---- tile -> core assignment (LPT on edge counts) ----
    t_of_e = dst // P
    tile_cnt = np.bincount(t_of_e, minlength=NTILES)
    order = np.argsort(-tile_cnt, kind="stable")
    core_loads = np.zeros(CORES, dtype=np.int64)
    core_tiles = [[] for _ in range(CORES)]
    for t in order:
        c = int(np.argmin(core_loads))
        core_tiles[c].append(int(t))
        core_loads[c] += tile_cnt[t]
    c_of_t = np.zeros(NTILES, dtype=np.int64)
    k_of_t = np.zeros(NTILES, dtype=np.int64)
    for c in range(CORES):
        for k, t in enumerate(core_tiles[c]):
            c_of_t[t] = c
            k_of_t[t] = k

    # table coordinates: node -> (table, row)
    node_ids = np.arange(NPAD)
    nc_core = c_of_t[node_ids // P]
    nc_slot = k_of_t[node_ids // P]
    tab_of = np.where(nc_slot < SLOT0[1], 0, np.where(nc_slot < SLOT0[2], 1, 2))
    slot_in_tab = nc_slot - np.array(SLOT0)[tab_of]
    rows_t = np.array(ROWS_T)[tab_of]
    trow = nc_core * rows_t + slot_in_tab * P + node_ids % P

    # ---- sort edges by (core, slot, table) ----
    e_tab = tab_of[src]
    e_row = trow[src]
    e_k = k_of_t[t_of_e]
    key = (c_of_t[t_of_e] * TPC + e_k) * NTAB + e_tab
    sort = np.argsort(key, kind="stable")
    row_s = e_row[sort]
    dstl_s = (dst % P).astype(np.float32)[sort]
    bounds = np.searchsorted(key[sort], np.arange(CORES * TPC * NTAB + 1))

    # ---- op schedule: per (group of G_MERGE slots, table) ----
    raw_ops = []   # (h, k0, k1, num_idxs, idxcol_off, [(j, k, paircol)...])
    idxcol_off = 0
    paircol = 0
    pairs_of_tile = np.zeros(TPC, dtype=np.int64)
    for k0 in range(0, TPC, G_MERGE):
        k1 = min(k0 + G_MERGE, TPC)
        for h in range(NTAB):
            m_op = [sum(int(bounds[(c * TPC + k) * NTAB + h + 1] -
                            bounds[(c * TPC + k) * NTAB + h])
                        for k in range(k0, k1)) for c in range(CORES)]
            n = rup(max(m_op), P) // P
            if n == 0:
                continue
            pairset = set()
            for c in range(CORES):
                off = 0
                for k in range(k0, k1):
                    g = (c * TPC + k) * NTAB + h
                    m = int(bounds[g + 1] - bounds[g])
                    if m == 0:
                        continue
                    for j in range(off // P, (off + m - 1) // P + 1):
                        pairset.add((j, k))
                    off += m
            pairlist = []
            # tile-major order: each tile's pairs contiguous -> <=2 live
            # PSUM banks per op regardless of G_MERGE
            for (k, j) in sorted((k, j) for (j, k) in pairset):
                pairlist.append((j, k, paircol))
                pairs_of_tile[k] += 1
                paircol += 1
            raw_ops.append((h, k0, k1, n * P, idxcol_off, pairlist))
            idxcol_off += n * P // 16
    GCOLS = idxcol_off
    NPAIRS = paircol
    MAXCH = max(op[3] // P for op in raw_ops)
    assert all(pairs_of_tile > 0)

    # staggered emission: stream per table, delays in rounds
    streams = [[op for op in raw_ops if op[0] == h] for h in range(NTAB)]
    ptr = [0] * NTAB
    ops_seq = []
    r = 0
    while any(ptr[h] < len(streams[h]) for h in range(NTAB)):
        for h in range(NTAB):
            if r >= DELAYS[h] and ptr[h] < len(streams[h]):
                ops_seq.append(streams[h][ptr[h]])
                ptr[h] += 1
        r += 1
    # move the final (tiny) T0 op to the very end: each conv then finishes
    # on a light-PE op, shrinking the PE tail drain and the cross-conv
    # boundary wait that tracks it
    last_a = max(i for i, op in enumerate(ops_seq) if op[0] == 0)
    ops_seq.append(ops_seq.pop(last_a))

    # ---- per-core arrays ----
    in_maps = []
    iota = np.tile(np.arange(P, dtype=np.float32)[None, :], (P, 1))
    emb_a_np = np.asarray(emb_a, dtype=np.float32)
    emb_b_np = np.asarray(emb_b, dtype=np.float32)
    for c in range(CORES):
        gidx16 = np.zeros((16, GCOLS), dtype=np.int16)
        dstlm = np.full((P, NPAIRS), PAD_DSTL, dtype=np.float32)
        for (h, k0, k1, num_idxs, coff, pairlist) in raw_ops:
            # pad with row 0 (gathered but zeroed by the PAD dstl columns):
            # trailing -1 trimming measured SLOWER (~10.6ns/idx) than just
            # gathering the padded row (~8.4ns/idx)
            idx = np.zeros(num_idxs, dtype=np.int16)
            tilearr = np.full(num_idxs, -1, dtype=np.int64)
            dl = np.full(num_idxs, PAD_DSTL, dtype=np.float32)
            off = 0
            for k in range(k0, k1):
                g = (c * TPC + k) * NTAB + h
                lo, hi = bounds[g], bounds[g + 1]
                m = int(hi - lo)
                if m == 0:
                    continue
                idx[off:off + m] = row_s[lo:hi].astype(np.int16)
                tilearr[off:off + m] = k
                dl[off:off + m] = dstl_s[lo:hi]
                off += m
            gidx16[:, coff:coff + num_idxs // 16] = wrap_idx(idx)
            for (j, k, pc_) in pairlist:
                seg_t = tilearr[j * P:(j + 1) * P]
                seg_d = dl[j * P:(j + 1) * P]
                dstlm[:, pc_] = np.where(seg_t == k, seg_d, PAD_DSTL)
        gidx = np.tile(gidx16, (8, 1))

        nodes = np.concatenate(
            [t * P + np.arange(P) for t in core_tiles[c]])
        valid = nodes < N
        nodes_c = np.where(valid, nodes, 0)
        x_own = np.where(valid[:, None], np.asarray(x)[nodes_c], 0.0).astype(np.float32)
        dinv_own = np.where(valid, dinv[nodes_c], 1.0).astype(np.float32)
        dinvc = dinv_own.reshape(TPC, P).T.copy()
        dinvr = np.sqrt(np.where(valid, deg[nodes_c], 1.0)).astype(
            ml_dtypes.bfloat16)[None, :]   # 1/dinv, [1, NLOC]

        xa = x_own[:, 0].astype(np.int64)
        xb = x_own[:, 1].astype(np.int64)
        fAT = np.ascontiguousarray(emb_a_np[xa].T).astype(ml_dtypes.bfloat16)
        fBT = np.ascontiguousarray(emb_b_np[xb].T).astype(ml_dtypes.bfloat16)
        x_ownT = np.ascontiguousarray(x_own.T[2:10]).astype(ml_dtypes.bfloat16)

        W1np = np.asarray(W1, dtype=np.float32)
        in_maps.append({
            "fAT": fAT,
            "fBT": fBT,
            "x_ownT": x_ownT,
            "W1a": W1np[0:64].astype(ml_dtypes.bfloat16),
            "W1b64": W1np[64:128].astype(ml_dtypes.bfloat16),
            "W1hi": W1np[128:136].astype(ml_dtypes.bfloat16),
            "W2": np.asarray(W2, dtype=np.float32),
            "b1row": np.asarray(b1, dtype=ml_dtypes.bfloat16)[None, :],
            "b2row": np.asarray(b2, dtype=ml_dtypes.bfloat16)[None, :],
            "dinvc": dinvc,
            "dinvr": dinvr,
            "iota": iota,
            "gidx": gidx,
            "dstlm": dstlm,
        })

    meta = {"raw_ops": raw_ops, "ops_seq": ops_seq, "GCOLS": GCOLS,
            "NPAIRS": NPAIRS, "MAXCH": MAXCH,
            "core_tiles": core_tiles}
    return in_maps, meta


def build(meta):
    ops_seq = meta["ops_seq"]
    GCOLS = meta["GCOLS"]
    NPAIRS = meta["NPAIRS"]
    MAXCH = meta["MAXCH"]
    f32 = mybir.dt.float32
    bf16 = mybir.dt.bfloat16
    i16 = mybir.dt.int16

    nc = bacc.Bacc("TRN2", target_bir_lowering=False, debug=False,
                   num_devices=CORES)
    fAT = nc.dram_tensor("fAT", [64, NLOC], bf16, kind="ExternalInput")
    fBT = nc.dram_tensor("fBT", [64, NLOC], bf16, kind="ExternalInput")
    x_ownT = nc.dram_tensor("x_ownT", [8, NLOC], bf16, kind="ExternalInput")
    W1a = nc.dram_tensor("W1a", [64, C1], bf16, kind="ExternalInput")
    W1b64 = nc.dram_tensor("W1b64", [64, C1], bf16, kind="ExternalInput")
    W1hi = nc.dram_tensor("W1hi", [8, C1], bf16, kind="ExternalInput")
    W2 = nc.dram_tensor("W2", [C1, C2], f32, kind="ExternalInput")
    b1row = nc.dram_tensor("b1row", [1, C1], bf16, kind="ExternalInput")
    b2row = nc.dram_tensor("b2row", [1, C2], bf16, kind="ExternalInput")
    dinvc = nc.dram_tensor("dinvc", [P, TPC], f32, kind="ExternalInput")
    dinvr = nc.dram_tensor("dinvr", [1, NLOC], bf16, kind="ExternalInput")
    iota = nc.dram_tensor("iota", [P, P], f32, kind="ExternalInput")
    gidx = nc.dram_tensor("gidx", [P, GCOLS], i16, kind="ExternalInput")
    dstlm = nc.dram_tensor("dstlm", [P, NPAIRS], f32, kind="ExternalInput")
    y = nc.dram_tensor("y", [NLOC, C2], f32, kind="ExternalOutput")

    with tile.TileContext(nc) as tc:
        with tc.tile_pool(name="const", bufs=1) as cpool, \
             tc.tile_pool(name="meta", bufs=1) as mpool, \
             tc.tile_pool(name="res", bufs=1) as rpool, \
             tc.tile_pool(name="he1", bufs=3) as he1pool, \
             tc.tile_pool(name="he2", bufs=3) as he2pool, \
             tc.tile_pool(name="feat", bufs=3) as fpool, \
             tc.tile_pool(name="dv", bufs=4) as dvpool, \
             tc.tile_pool(name="sel", bufs=4) as spool, \
             tc.tile_pool(name="epi", bufs=2) as tpool, \
             tc.tile_pool(name="ptr", bufs=2, space="PSUM") as ptrp, \
             tc.tile_pool(name="pmm", bufs=1, space="PSUM") as pmmp, \
             tc.tile_pool(name="pacc", bufs=5, space="PSUM") as paccp, \
             tc.tile_pool(name="dram", bufs=1, space="DRAM") as dram:

            # ---------- constants ----------
            ident = cpool.tile([P, P], f32, tag="ident")
            make_identity(nc, ident[:])
            identb = cpool.tile([P, P], bf16, tag="identb")
            nc.vector.tensor_copy(out=identb[:], in_=ident[:])
            iota_t = cpool.tile([P, P], f32, tag="iota")
            nc.sync.dma_start(out=iota_t[:], in_=iota[:])
            W1at = cpool.tile([64, C1], bf16, tag="w1a")
            nc.sync.dma_start(out=W1at[:], in_=W1a[:])
            W1bt = cpool.tile([64, C1], bf16, tag="w1b")
            nc.sync.dma_start(out=W1bt[:], in_=W1b64[:])
            W1ht = cpool.tile([8, C1], bf16, tag="w1h")
            nc.sync.dma_start(out=W1ht[:], in_=W1hi[:])
            W2t = cpool.tile([C1, C2], f32, tag="w2")
            nc.sync.dma_start(out=W2t[:], in_=W2[:])
            b1t = cpool.tile([1, C1], bf16, tag="b1")
            nc.sync.dma_start(out=b1t[:], in_=b1row[:])
            b2t = cpool.tile([1, C2], bf16, tag="b2")
            nc.sync.dma_start(out=b2t[:], in_=b2row[:])
            dinv_t = cpool.tile([P, TPC], f32, tag="dinv")
            nc.sync.dma_start(out=dinv_t[:], in_=dinvc[:])
            gidx_t = mpool.tile([P, GCOLS], i16, tag="gidx")
            nc.sync.dma_start(out=gidx_t[:], in_=gidx[:])
            dstl_t = mpool.tile([P, NPAIRS], f32, tag="dstl")
            nc.sync.dma_start(out=dstl_t[:], in_=dstlm[:])

            ag1 = [dram.tile([ROWS_T[t], C1], bf16, tag=f"ag1_{t}",
                             name=f"ag1_{t}") for t in range(NTAB)]
            tab1 = [dram.tile([TROWS[t], C1], bf16, tag=f"tab1_{t}",
                              name=f"tab1_{t}") for t in range(NTAB)]
            ag2 = [dram.tile([ROWS_T[t], C2], f32, tag=f"ag2_{t}",
                             name=f"ag2_{t}") for t in range(NTAB)]
            tab2 = [dram.tile([TROWS[t], C2], f32, tag=f"tab2_{t}",
                              name=f"tab2_{t}") for t in range(NTAB)]

            def slot_dst(k, tabs):
                t = tab_of_slot(k)
                return tabs[t], (k - SLOT0[t]) * P

            # resident per-tile h1s (bf16) and h2s (f32) for self-loop terms
            h1s_res = [rpool.tile([P, C1], bf16, tag=f"h1s_{k}",
                                  name=f"h1s_{k}") for k in range(TPC)]
            h2s_res = [rpool.tile([P, C2], f32, tag=f"h2s_{k}",
                                  name=f"h2s_{k}") for k in range(TPC)]
            acc1 = [rpool.tile([P, C1], f32, tag=f"acc1_{k}",
                               name=f"acc1_{k}") for k in range(TPC)]
            acc2 = [rpool.tile([P, C2], f32, tag=f"acc2_{k}",
                               name=f"acc2_{k}") for k in range(TPC)]

            # ---------- stage 1: h1 = feat @ W1, scaled by dinv ----------
            for k in range(TPC):
                sl = slice(k * P, (k + 1) * P)
                fa = fpool.tile([64, P], bf16, tag="fa", name=f"fa_{k}")
                nc.sync.dma_start(out=fa[:], in_=fAT[:, sl])
                fb = fpool.tile([64, P], bf16, tag="fb", name=f"fb_{k}")
                nc.sync.dma_start(out=fb[:], in_=fBT[:, sl])
                fx = fpool.tile([8, P], bf16, tag="fx", name=f"fx_{k}")
                nc.sync.dma_start(out=fx[:], in_=x_ownT[:, sl])
                ph1 = pmmp.tile([P, C1], f32, space="PSUM", tag="pmm",
                                name=f"ph1_{k}")
                nc.tensor.matmul(out=ph1[:], lhsT=fa[:], rhs=W1at[:],
                                 start=True, stop=False)
                nc.tensor.matmul(out=ph1[:], lhsT=fb[:], rhs=W1bt[:],
                                 start=False, stop=False)
                nc.tensor.matmul(out=ph1[:], lhsT=fx[:], rhs=W1ht[:],
                                 start=False, stop=True)
                h1s = h1s_res[k]
                nc.scalar.activation(out=h1s[:], in_=ph1[:],
                                     func=mybir.ActivationFunctionType.Copy,
                                     scale=dinv_t[:, k:k + 1])
                dstt, off = slot_dst(k, ag1)
                nc.sync.dma_start(out=dstt[off:off + P, :], in_=h1s[:])
                # fire table 0's AllGather as soon as its last slot lands;
                # tables 1/2 are deferred into the conv1 op stream so the Q7
                # queue is not serialized behind their input DMAs
                if k == SLOT0[0] + SLOTS[0] - 1:
                    nc.gpsimd.collective_compute(
                        "AllGather", mybir.AluOpType.bypass,
                        replica_groups=[list(range(CORES))],
                        ins=[ag1[0].opt()], outs=[tab1[0].opt()])

            # ---------- conv pass ----------
            def conv(tabs, slot_dst_tabs, C, hepool, hetag, hedt, Sdt,
                     btile, last, agg_next=None, defers=None):
                acc = acc1 if C == C1 else acc2
                hres = h1s_res if C == C1 else h2s_res
                identsl = identb if C == C1 else ident
                npairs_of = {h: {} for h in range(NTAB)}
                for op in ops_seq:
                    for (j, k, pc_) in op[5]:
                        d = npairs_of[op[0]]
                        d[k] = d.get(k, 0) + 1
                done_of = {h: {k: 0 for k in npairs_of[h]} for h in range(NTAB)}
                phases_of = {k: sum(1 for h in range(NTAB)
                                    if npairs_of[h].get(k, 0) > 0)
                             for k in range(TPC)}
                phases_done = {k: 0 for k in range(TPC)}
                bank_of = {}

                def epilogue(k, pacc_ap):
                    # final sum: sbuf acc + last-phase psum (incl self+bias)
                    t1 = tpool.tile([P, C], f32, tag=f"t1{C}",
                                    name=f"t1_{C}_{k}")
                    if phases_of[k] > 1:
                        nc.vector.tensor_add(out=t1[:], in0=acc[k][:],
                                             in1=pacc_ap)
                        src_ap = t1[:]
                    else:
                        src_ap = pacc_ap
                    hrelu = tpool.tile([P, C], f32, tag=f"hr{C}",
                                       name=f"hr_{C}_{k}")
                    nc.scalar.activation(out=hrelu[:], in_=src_ap,
                                         func=mybir.ActivationFunctionType.Relu,
                                         scale=dinv_t[:, k:k + 1])
                    if not last:
                        ptr2 = ptrp.tile([P, P], f32, space="PSUM", tag="ptr",
                                         name=f"ptr2_{k}")
                        nc.tensor.transpose(out=ptr2[:], in_=hrelu[:],
                                            identity=ident[:])
                        hT = tpool.tile([P, P], f32, tag="hT", name=f"hT_{k}")
                        nc.scalar.activation(
                            out=hT[:], in_=ptr2[:],
                            func=mybir.ActivationFunctionType.Copy)
                        ph2 = pmmp.tile([P, C2], f32, space="PSUM", tag="pmm",
                                        name=f"ph2_{k}")
                        nc.tensor.matmul(out=ph2[:], lhsT=hT[:], rhs=W2t[:],
                                         start=True, stop=True)
                        h2s = h2s_res[k]
                        nc.scalar.activation(
                            out=h2s[:], in_=ph2[:],
                            func=mybir.ActivationFunctionType.Copy,
                            scale=dinv_t[:, k:k + 1])
                        d2, o2 = slot_dst(k, ag2)
                        nc.sync.dma_start(out=d2[o2:o2 + P, :], in_=h2s[:])
                        if agg_next is not None:
                            agg_next(k)
                    else:
                        nc.sync.dma_start(out=y[k * P:(k + 1) * P, :],
                                          in_=hrelu[:])

                for opi, (h, k0, k1, num_idxs, coff, pairlist) in \
                        enumerate(ops_seq):
                    if defers and opi in defers:
                        defers.pop(opi)()
                    he = hepool.tile([P, MAXCH * C], hedt, tag=hetag,
                                     name=f"he_{C}_{h}_{k0}")
                    nch = num_idxs // P
                    nc.gpsimd.dma_gather(
                        out_ap=he[:, 0:nch * C].rearrange(
                            "p (n c) -> p n c", c=C),
                        in_ap=tabs[h][:],
                        idxs_ap=gidx_t[:, coff:coff + num_idxs // 16],
                        num_idxs=num_idxs, num_idxs_reg=num_idxs, elem_size=C,
                        single_packet=(num_idxs <= 1024))
                    for (j, k, pc_) in pairlist:
                        if k not in bank_of:
                            bank_of[k] = paccp.tile(
                                [P, C], f32, space="PSUM",
                                tag="pacc", name=f"pacc_{C}_{h}_{k}")
                        pacc_ap = bank_of[k][:]
                        S = spool.tile([P, P], Sdt, tag=f"S{C}",
                                       name=f"S_{C}_{pc_}")
                        nc.vector.tensor_tensor(
                            out=S[:],
                            in0=dstl_t[:, pc_:pc_ + 1].to_broadcast([P, P]),
                            in1=iota_t[:],
                            op=mybir.AluOpType.is_equal)
                        is_first = done_of[h][k] == 0
                        is_last_pair = done_of[h][k] == npairs_of[h][k] - 1
                        last_phase = (phases_done[k] == phases_of[k] - 1)
                        nc.tensor.matmul(out=pacc_ap, lhsT=S[:],
                                         rhs=he[:, j * C:(j + 1) * C],
                                         start=is_first,
                                         stop=(is_last_pair and not last_phase))
                        done_of[h][k] += 1
                        if done_of[h][k] == npairs_of[h][k]:
                            bank = bank_of.pop(k)
                            phases_done[k] += 1
                            if phases_done[k] < phases_of[k]:
                                # spill partial into sbuf accumulator
                                if phases_done[k] == 1:
                                    nc.scalar.activation(
                                        out=acc[k][:], in_=bank[:],
                                        func=mybir.ActivationFunctionType.Copy)
                                else:
                                    nc.vector.tensor_add(
                                        out=acc[k][:], in0=acc[k][:],
                                        in1=bank[:])
                            else:
                                # last phase: fold self-loop + bias into psum
                                nc.tensor.matmul(
                                    out=bank[:], lhsT=identsl[:],
                                    rhs=hres[k][:], start=False, stop=False)
                                dv = dvpool.tile([1, P], bf16, tag="dv",
                                                 name=f"dv_{C}_{k}")
                                nc.sync.dma_start(
                                    out=dv[:], in_=dinvr[:, k * P:(k + 1) * P])
                                nc.tensor.matmul(
                                    out=bank[:], lhsT=dv[:],
                                    rhs=btile[:], start=False, stop=True)
                                epilogue(k, bank[:])

                for hh in range(NTAB):
                    assert all(done_of[hh][k] == npairs_of[hh][k]
                               for k in npairs_of[hh])

            fired = set()

            def agg_next(k):
                for t in range(NTAB):
                    if k == SLOT0[t] + SLOTS[t] - 1 and t != NTAB - 1:
                        fired.add(t)
                        nc.gpsimd.collective_compute(
                            "AllGather", mybir.AluOpType.bypass,
                            replica_groups=[list(range(CORES))],
                            ins=[ag2[t].opt()], outs=[tab2[t].opt()])

            def defer_last_ag2():
                nc.gpsimd.collective_compute(
                    "AllGather", mybir.AluOpType.bypass,
                    replica_groups=[list(range(CORES))],
                    ins=[ag2[NTAB - 1].opt()], outs=[tab2[NTAB - 1].opt()])

            def mk_ag1(t):
                def fire():
                    nc.gpsimd.collective_compute(
                        "AllGather", mybir.AluOpType.bypass,
                        replica_groups=[list(range(CORES))],
                        ins=[ag1[t].opt()], outs=[tab1[t].opt()])
                return fire

            conv(tab1, ag1, C1, he1pool, "he1", bf16, bf16, b1t,
                 last=False, agg_next=agg_next,
                 defers={1: mk_ag1(1), 2: mk_ag1(2)})
            conv(tab2, ag2, C2, he2pool, "he2", f32, f32, b2t,
                 last=True, defers={5: defer_last_ag2})

    nc.compile()
    return nc


_cache = {}


def kernel(x, edge_index, emb_a, emb_b, W1, b1, W2, b2):
    in_maps, meta = prep(x, edge_index, emb_a, emb_b, W1, b1, W2, b2)
    key = (meta["GCOLS"], meta["NPAIRS"], meta["MAXCH"],
           tuple((op[0], op[1], op[2], op[3], op[4], tuple(op[5]))
                 for op in meta["ops_seq"]))
    if key not in _cache:
        _cache[key] = build(meta)
    nc = _cache[key]
    res = run_bass_kernel_spmd(nc, in_maps, core_ids=list(range(CORES)))
    out = np.zeros((N, C2), dtype=np.float32)
    for c in range(CORES):
        yc = res.results[c]["y"]
        nodes = np.concatenate(
            [t * P + np.arange(P) for t in meta["core_tiles"][c]])
        valid = nodes < N
        out[nodes[valid]] = yc[valid]
    return out


# revision 18
# speedup vs baseline: 1.2174x; 1.2174x over previous
"""Self-contained GCN encoder kernel for 8 TRN2 NeuronCores (Bass/Tile).

kernel(**inputs) takes the FULL unsharded inputs (as from setup_inputs())
and returns the FULL [50000, 64] float32 output.

Strategy: dst-node tiles of 128 are LPT-balanced across 8 cores; the
NEFF is specialized per run to the edge structure. Embedding lookups are
done host-side and shipped channel-major; stage 1 is pure matmul with
per-tile feature DMAs. Each conv's node table is split into THREE
sub-tables ([8,20,21] slots x 128 x 8 cores) so the first AllGather fires
after only 8 stage-1 tiles; the remaining stage-1 tiles and the other
AllGather triggers are deferred into the conv-1 op stream so neither the
Q7 queue nor the PE queue serializes behind them. Per (8-dst-tile group,
sub-table) gather ops use dma_gather (the Q7 descriptor-generation rate
of ~8ns/row is the kernel's roofline; the Pool engine runs >98% busy
between first and last gather). Segment-reduction is on the TensorEngine
via is_equal selection matrices into PSUM (tile-major pair order keeps
<=2 banks live per op) with per-phase SBUF spills; the self-loop term is
an identity matmul of SBUF-resident h tiles and the bias is a rank-1
((1/dinv) x b) matmul, so the epilogue is one DVE add + one fused
Relu-scale activation on the Scalar engine (keeps the DVE queue free of
head-of-line stalls). The six AllGathers are staggered via per-table
op-stream delays so neither conv ever waits on a collective.
"""
import numpy as np
import ml_dtypes
from concourse import bass, bacc, mybir, tile
from concourse.bass_utils import run_bass_kernel_spmd
from concourse.masks import make_identity

P = 128
CORES = 8
N = 50000
NTILES = 392
NPAD = NTILES * P     # 50176
TPC = NTILES // CORES  # 49
NLOC = TPC * P        # 6272
SLOTS = [8, 20, 21]   # slots per sub-table (sum = TPC)
SLOT0 = [0, 8, 28]    # first slot of each sub-table
NTAB = 3
ROWS_T = [s * P for s in SLOTS]          # per-core rows per table
TROWS = [CORES * r for r in ROWS_T]      # total rows per table
C1 = 128
C2 = 64
IN_CH = 136
PAD_DSTL = 30000.0
G_MERGE = 8
NGROUPS = (TPC + G_MERGE - 1) // G_MERGE
DELAYS = [0, 3, 5]    # stagger (in rounds) of the three table streams


def wrap_idx(arr):
    return arr.reshape(-1, 16).T


def rup(x, m):
    return int((x + m - 1) // m * m)


def tab_of_slot(k):
    if k < SLOT0[1]:
        return 0
    if k < SLOT0[2]:
        return 1
    return 2


def prep(x, edge_index, emb_a, emb_b, W1, b1, W2, b2):
    src, dst = np.asarray(edge_index[0]), np.asarray(edge_index[1])
    deg = np.bincount(dst, minlength=N).astype(np.float32) + 1.0
    dinv = (1.0 / np.sqrt(deg)).astype(np.float32)

    # ---- tile -> core assignment (LPT on edge counts) ----
    t_of_e = dst // P
    tile_cnt = np.bincount(t_of_e, minlength=NTILES)
    order = np.argsort(-tile_cnt, kind="stable")
    core_loads = np.zeros(CORES, dtype=np.int64)
    core_tiles = [[] for _ in range(CORES)]
    for t in order:
        c = int(np.argmin(core_loads))
        core_tiles[c].append(int(t))
        core_loads[c] += tile_cnt[t]
    c_of_t = np.zeros(NTILES, dtype=np.int64)
    k_of_t = np.zeros(NTILES, dtype=np.int64)
    for c in range(CORES):
        for k, t in enumerate(core_tiles[c]):
            c_of_t[t] = c
            k_of_t[t] = k

    # table coordinates: node -> (table, row)
    node_ids = np.arange(NPAD)
    nc_core = c_of_t[node_ids // P]
    nc_slot = k_of_t[node_ids // P]
    tab_of = np.where(nc_slot < SLOT0[1], 0, np.where(nc_slot < SLOT0[2], 1, 2))
    slot_in_tab = nc_slot - np.array(SLOT0)[tab_of]
    rows_t = np.array(ROWS_T)[tab_of]
    trow = nc_core * rows_t + slot_in_tab * P + node_ids % P

    # ---- sort edges by (core, slot, table) ----
    e_tab = tab_of[src]
    e_row = trow[src]
    e_k = k_of_t[t_of_e]
    key = (c_of_t[t_of_e] * TPC + e_k) * NTAB + e_tab
    sort = np.argsort(key, kind="stable")
    row_s = e_row[sort]
    dstl_s = (dst % P).astype(np.float32)[sort]
    bounds = np.searchsorted(key[sort], np.arange(CORES * TPC * NTAB + 1))

    # ---- op schedule: per (group of G_MERGE slots, table) ----
    raw_ops = []   # (h, k0, k1, num_idxs, idxcol_off, [(j, k, paircol)...])
    idxcol_off = 0
    paircol = 0
    pairs_of_tile = np.zeros(TPC, dtype=np.int64)
    for k0 in range(0, TPC, G_MERGE):
        k1 = min(k0 + G_MERGE, TPC)
        for h in range(NTAB):
            m_op = [sum(int(bounds[(c * TPC + k) * NTAB + h + 1] -
                            bounds[(c * TPC + k) * NTAB + h])
                        for k in range(k0, k1)) for c in range(CORES)]
            n = rup(max(m_op), P) // P
            if n == 0:
                continue
            pairset = set()
            for c in range(CORES):
                off = 0
                for k in range(k0, k1):
                    g = (c * TPC + k) * NTAB + h
                    m = int(bounds[g + 1] - bounds[g])
                    if m == 0:
                        continue
                    for j in range(off // P, (off + m - 1) // P + 1):
                        pairset.add((j, k))
                    off += m
            pairlist = []
            # tile-major order: each tile's pairs contiguous -> <=2 live
            # PSUM banks per op regardless of G_MERGE
            for (k, j) in sorted((k, j) for (j, k) in pairset):
                pairlist.append((j, k, paircol))
                pairs_of_tile[k] += 1
                paircol += 1
            raw_ops.append((h, k0, k1, n * P, idxcol_off, pairlist))
            idxcol_off += n * P // 16
    GCOLS = idxcol_off
    NPAIRS = paircol
    MAXCH = max(op[3] // P for op in raw_ops)
    assert all(pairs_of_tile > 0)

    # staggered emission: stream per table, delays in rounds
    streams = [[op for op in raw_ops if op[0] == h] for h in range(NTAB)]
    ptr = [0] * NTAB
    ops_seq = []
    r = 0
    while any(ptr[h] < len(streams[h]) for h in range(NTAB)):
        for h in range(NTAB):
            if r >= DELAYS[h] and ptr[h] < len(streams[h]):
                ops_seq.append(streams[h][ptr[h]])
                ptr[h] += 1
        r += 1

    # ---- per-core arrays ----
    in_maps = []
    iota = np.tile(np.arange(P, dtype=np.float32)[None, :], (P, 1))
    emb_a_np = np.asarray(emb_a, dtype=np.float32)
    emb_b_np = np.asarray(emb_b, dtype=np.float32)
    for c in range(CORES):
        gidx16 = np.zeros((16, GCOLS), dtype=np.int16)
        dstlm = np.full((P, NPAIRS), PAD_DSTL, dtype=np.float32)
        for (h, k0, k1, num_idxs, coff, pairlist) in raw_ops:
            # pad with row 0 (gathered but zeroed by the PAD dstl columns):
            # trailing -1 trimming measured SLOWER (~10.6ns/idx) than just
            # gathering the padded row (~8.4ns/idx)
            idx = np.zeros(num_idxs, dtype=np.int16)
            tilearr = np.full(num_idxs, -1, dtype=np.int64)
            dl = np.full(num_idxs, PAD_DSTL, dtype=np.float32)
            off = 0
            for k in range(k0, k1):
                g = (c * TPC + k) * NTAB + h
                lo, hi = bounds[g], bounds[g + 1]
                m = int(hi - lo)
                if m == 0:
                    continue
                idx[off:off + m] = row_s[lo:hi].astype(np.int16)
                tilearr[off:off + m] = k
                dl[off:off + m] = dstl_s[lo:hi]
                off += m
            gidx16[:, coff:coff + num_idxs // 16] = wrap_idx(idx)
            for (j, k, pc_) in pairlist:
                seg_t = tilearr[j * P:(j + 1) * P]
                seg_d = dl[j * P:(j + 1) * P]
                dstlm[:, pc_] = np.where(seg_t == k, seg_d, PAD_DSTL)
        gidx = np.tile(gidx16, (8, 1))

        nodes = np.concatenate(
            [t * P + np.arange(P) for t in core_tiles[c]])
        valid = nodes < N
        nodes_c = np.where(valid, nodes, 0)
        x_own = np.where(valid[:, None], np.asarray(x)[nodes_c], 0.0).astype(np.float32)
        dinv_own = np.where(valid, dinv[nodes_c], 1.0).astype(np.float32)
        dinvc = dinv_own.reshape(TPC, P).T.copy()
        dinvr = np.sqrt(np.where(valid, deg[nodes_c], 1.0)).astype(
            ml_dtypes.bfloat16)[None, :]   # 1/dinv, [1, NLOC]

        xa = x_own[:, 0].astype(np.int64)
        xb = x_own[:, 1].astype(np.int64)
        fAT = np.ascontiguousarray(emb_a_np[xa].T).astype(ml_dtypes.bfloat16)
        fBT = np.ascontiguousarray(emb_b_np[xb].T).astype(ml_dtypes.bfloat16)
        x_ownT = np.ascontiguousarray(x_own.T[2:10]).astype(ml_dtypes.bfloat16)

        W1np = np.asarray(W1, dtype=np.float32)
        in_maps.append({
            "fAT": fAT,
            "fBT": fBT,
            "x_ownT": x_ownT,
            "W1a": W1np[0:64].astype(ml_dtypes.bfloat16),
            "W1b64": W1np[64:128].astype(ml_dtypes.bfloat16),
            "W1hi": W1np[128:136].astype(ml_dtypes.bfloat16),
            "W2": np.asarray(W2, dtype=np.float32),
            "b1row": np.asarray(b1, dtype=ml_dtypes.bfloat16)[None, :],
            "b2row": np.asarray(b2, dtype=ml_dtypes.bfloat16)[None, :],
            "dinvc": dinvc,
            "dinvr": dinvr,
            "iota": iota,
            "gidx": gidx,
            "dstlm": dstlm,
        })

    meta = {"raw_ops": raw_ops, "ops_seq": ops_seq, "GCOLS": GCOLS,
            "NPAIRS": NPAIRS, "MAXCH": MAXCH,
            "core_tiles": core_tiles}
    return in_maps, meta


def build(meta):
    ops_seq = meta["ops_seq"]
    GCOLS = meta["GCOLS"]
    NPAIRS = meta["NPAIRS"]
    MAXCH = meta["MAXCH"]
    f32 = mybir.dt.float32
    bf16 = mybir.dt.bfloat16
    i16 = mybir.dt.int16

    nc = bacc.Bacc("TRN2", target_bir_lowering=False, debug=False,
                   num_devices=CORES)
    fAT = nc.dram_tensor("fAT", [64, NLOC], bf16, kind="ExternalInput")
    fBT = nc.dram_tensor("fBT", [64, NLOC], bf16, kind="ExternalInput")
    x_ownT = nc.dram_tensor("x_ownT", [8, NLOC], bf16, kind="ExternalInput")
    W1a = nc.dram_tensor("W1a", [64, C1], bf16, kind="ExternalInput")
    W1b64 = nc.dram_tensor("W1b64", [64, C1], bf16, kind="ExternalInput")
    W1hi = nc.dram_tensor("W1hi", [8, C1], bf16, kind="ExternalInput")
    W2 = nc.dram_tensor("W2", [C1, C2], f32, kind="ExternalInput")
    b1row = nc.dram_tensor("b1row", [1, C1], bf16, kind="ExternalInput")
    b2row = nc.dram_tensor("b2row", [1, C2], bf16, kind="ExternalInput")
    dinvc = nc.dram_tensor("dinvc", [P, TPC], f32, kind="ExternalInput")
    dinvr = nc.dram_tensor("dinvr", [1, NLOC], bf16, kind="ExternalInput")
    iota = nc.dram_tensor("iota", [P, P], f32, kind="ExternalInput")
    gidx = nc.dram_tensor("gidx", [P, GCOLS], i16, kind="ExternalInput")
    dstlm = nc.dram_tensor("dstlm", [P, NPAIRS], f32, kind="ExternalInput")
    y = nc.dram_tensor("y", [NLOC, C2], f32, kind="ExternalOutput")

    with tile.TileContext(nc) as tc:
        with tc.tile_pool(name="const", bufs=1) as cpool, \
             tc.tile_pool(name="meta", bufs=1) as mpool, \
             tc.tile_pool(name="res", bufs=1) as rpool, \
             tc.tile_pool(name="he1", bufs=3) as he1pool, \
             tc.tile_pool(name="he2", bufs=3) as he2pool, \
             tc.tile_pool(name="feat", bufs=3) as fpool, \
             tc.tile_pool(name="dv", bufs=4) as dvpool, \
             tc.tile_pool(name="sel", bufs=4) as spool, \
             tc.tile_pool(name="epi", bufs=2) as tpool, \
             tc.tile_pool(name="ptr", bufs=2, space="PSUM") as ptrp, \
             tc.tile_pool(name="pmm", bufs=1, space="PSUM") as pmmp, \
             tc.tile_pool(name="pacc", bufs=5, space="PSUM") as paccp, \
             tc.tile_pool(name="dram", bufs=1, space="DRAM") as dram:

            # ---------- constants ----------
            ident = cpool.tile([P, P], f32, tag="ident")
            make_identity(nc, ident[:])
            identb = cpool.tile([P, P], bf16, tag="identb")
            nc.vector.tensor_copy(out=identb[:], in_=ident[:])
            iota_t = cpool.tile([P, P], f32, tag="iota")
            nc.sync.dma_start(out=iota_t[:], in_=iota[:])
            W1at = cpool.tile([64, C1], bf16, tag="w1a")
            nc.sync.dma_start(out=W1at[:], in_=W1a[:])
            W1bt = cpool.tile([64, C1], bf16, tag="w1b")
            nc.sync.dma_start(out=W1bt[:], in_=W1b64[:])
            W1ht = cpool.tile([8, C1], bf16, tag="w1h")
            nc.sync.dma_start(out=W1ht[:], in_=W1hi[:])
            W2t = cpool.tile([C1, C2], f32, tag="w2")
            nc.sync.dma_start(out=W2t[:], in_=W2[:])
            b1t = cpool.tile([1, C1], bf16, tag="b1")
            nc.sync.dma_start(out=b1t[:], in_=b1row[:])
            b2t = cpool.tile([1, C2], bf16, tag="b2")
            nc.sync.dma_start(out=b2t[:], in_=b2row[:])
            dinv_t = cpool.tile([P, TPC], f32, tag="dinv")
            nc.sync.dma_start(out=dinv_t[:], in_=dinvc[:])
            gidx_t = mpool.tile([P, GCOLS], i16, tag="gidx")
            nc.sync.dma_start(out=gidx_t[:], in_=gidx[:])
            dstl_t = mpool.tile([P, NPAIRS], f32, tag="dstl")
            nc.sync.dma_start(out=dstl_t[:], in_=dstlm[:])

            ag1 = [dram.tile([ROWS_T[t], C1], bf16, tag=f"ag1_{t}",
                             name=f"ag1_{t}") for t in range(NTAB)]
            tab1 = [dram.tile([TROWS[t], C1], bf16, tag=f"tab1_{t}",
                              name=f"tab1_{t}") for t in range(NTAB)]
            ag2 = [dram.tile([ROWS_T[t], C2], f32, tag=f"ag2_{t}",
                             name=f"ag2_{t}") for t in range(NTAB)]
            tab2 = [dram.tile([TROWS[t], C2], f32, tag=f"tab2_{t}",
                              name=f"tab2_{t}") for t in range(NTAB)]

            def slot_dst(k, tabs):
                t = tab_of_slot(k)
                return tabs[t], (k - SLOT0[t]) * P

            # resident per-tile h1s (bf16) and h2s (f32) for self-loop terms
            h1s_res = [rpool.tile([P, C1], bf16, tag=f"h1s_{k}",
                                  name=f"h1s_{k}") for k in range(TPC)]
            h2s_res = [rpool.tile([P, C2], f32, tag=f"h2s_{k}",
                                  name=f"h2s_{k}") for k in range(TPC)]
            acc1 = [rpool.tile([P, C1], f32, tag=f"acc1_{k}",
                               name=f"acc1_{k}") for k in range(TPC)]
            acc2 = [rpool.tile([P, C2], f32, tag=f"acc2_{k}",
                               name=f"acc2_{k}") for k in range(TPC)]

            # ---------- stage 1: h1 = feat @ W1, scaled by dinv ----------
            for k in range(TPC):
                sl = slice(k * P, (k + 1) * P)
                fa = fpool.tile([64, P], bf16, tag="fa", name=f"fa_{k}")
                nc.sync.dma_start(out=fa[:], in_=fAT[:, sl])
                fb = fpool.tile([64, P], bf16, tag="fb", name=f"fb_{k}")
                nc.sync.dma_start(out=fb[:], in_=fBT[:, sl])
                fx = fpool.tile([8, P], bf16, tag="fx", name=f"fx_{k}")
                nc.sync.dma_start(out=fx[:], in_=x_ownT[:, sl])
                ph1 = pmmp.tile([P, C1], f32, space="PSUM", tag="pmm",
                                name=f"ph1_{k}")
                nc.tensor.matmul(out=ph1[:], lhsT=fa[:], rhs=W1at[:],
                                 start=True, stop=False)
                nc.tensor.matmul(out=ph1[:], lhsT=fb[:], rhs=W1bt[:],
                                 start=False, stop=False)
                nc.tensor.matmul(out=ph1[:], lhsT=fx[:], rhs=W1ht[:],
                                 start=False, stop=True)
                h1s = h1s_res[k]
                nc.scalar.activation(out=h1s[:], in_=ph1[:],
                                     func=mybir.ActivationFunctionType.Copy,
                                     scale=dinv_t[:, k:k + 1])
                dstt, off = slot_dst(k, ag1)
                nc.sync.dma_start(out=dstt[off:off + P, :], in_=h1s[:])
                # fire table 0's AllGather as soon as its last slot lands;
                # tables 1/2 are deferred into the conv1 op stream so the Q7
                # queue is not serialized behind their input DMAs
                if k == SLOT0[0] + SLOTS[0] - 1:
                    nc.gpsimd.collective_compute(
                        "AllGather", mybir.AluOpType.bypass,
                        replica_groups=[list(range(CORES))],
                        ins=[ag1[0].opt()], outs=[tab1[0].opt()])

            # ---------- conv pass ----------
            def conv(tabs, slot_dst_tabs, C, hepool, hetag, hedt, Sdt,
                     btile, last, agg_next=None, defers=None):
                acc = acc1 if C == C1 else acc2
                hres = h1s_res if C == C1 else h2s_res
                identsl = identb if C == C1 else ident
                npairs_of = {h: {} for h in range(NTAB)}
                for op in ops_seq:
                    for (j, k, pc_) in op[5]:
                        d = npairs_of[op[0]]
                        d[k] = d.get(k, 0) + 1
                done_of = {h: {k: 0 for k in npairs_of[h]} for h in range(NTAB)}
                phases_of = {k: sum(1 for h in range(NTAB)
                                    if npairs_of[h].get(k, 0) > 0)
                             for k in range(TPC)}
                phases_done = {k: 0 for k in range(TPC)}
                bank_of = {}

                def epilogue(k, pacc_ap):
                    # final sum: sbuf acc + last-phase psum (incl self+bias)
                    t1 = tpool.tile([P, C], f32, tag=f"t1{C}",
                                    name=f"t1_{C}_{k}")
                    if phases_of[k] > 1:
                        nc.vector.tensor_add(out=t1[:], in0=acc[k][:],
                                             in1=pacc_ap)
                        src_ap = t1[:]
                    else:
                        src_ap = pacc_ap
                    hrelu = tpool.tile([P, C], f32, tag=f"hr{C}",
                                       name=f"hr_{C}_{k}")
                    nc.scalar.activation(out=hrelu[:], in_=src_ap,
                                         func=mybir.ActivationFunctionType.Relu,
                                         scale=dinv_t[:, k:k + 1])
                    if not last:
                        ptr2 = ptrp.tile([P, P], f32, space="PSUM", tag="ptr",
                                         name=f"ptr2_{k}")
                        nc.tensor.transpose(out=ptr2[:], in_=hrelu[:],
                                            identity=ident[:])
                        hT = tpool.tile([P, P], f32, tag="hT", name=f"hT_{k}")
                        nc.scalar.activation(
                            out=hT[:], in_=ptr2[:],
                            func=mybir.ActivationFunctionType.Copy)
                        ph2 = pmmp.tile([P, C2], f32, space="PSUM", tag="pmm",
                                        name=f"ph2_{k}")
                        nc.tensor.matmul(out=ph2[:], lhsT=hT[:], rhs=W2t[:],
                                         start=True, stop=True)
                        h2s = h2s_res[k]
                        nc.scalar.activation(
                            out=h2s[:], in_=ph2[:],
                            func=mybir.ActivationFunctionType.Copy,
                            scale=dinv_t[:, k:k + 1])
                        d2, o2 = slot_dst(k, ag2)
                        nc.sync.dma_start(out=d2[o2:o2 + P, :], in_=h2s[:])
                        if agg_next is not None:
                            agg_next(k)
                    else:
                        nc.sync.dma_start(out=y[k * P:(k + 1) * P, :],
                                          in_=hrelu[:])

                for opi, (h, k0, k1, num_idxs, coff, pairlist) in \
                        enumerate(ops_seq):
                    if defers and opi in defers:
                        defers.pop(opi)()
                    he = hepool.tile([P, MAXCH * C], hedt, tag=hetag,
                                     name=f"he_{C}_{h}_{k0}")
                    nch = num_idxs // P
                    nc.gpsimd.dma_gather(
                        out_ap=he[:, 0:nch * C].rearrange(
                            "p (n c) -> p n c", c=C),
                        in_ap=tabs[h][:],
                        idxs_ap=gidx_t[:, coff:coff + num_idxs // 16],
                        num_idxs=num_idxs, num_idxs_reg=num_idxs, elem_size=C,
                        single_packet=(num_idxs <= 1024))
                    for (j, k, pc_) in pairlist:
                        if k not in bank_of:
                            bank_of[k] = paccp.tile(
                                [P, C], f32, space="PSUM",
                                tag="pacc", name=f"pacc_{C}_{h}_{k}")
                        pacc_ap = bank_of[k][:]
                        S = spool.tile([P, P], Sdt, tag=f"S{C}",
                                       name=f"S_{C}_{pc_}")
                        nc.vector.tensor_tensor(
                            out=S[:],
                            in0=dstl_t[:, pc_:pc_ + 1].to_broadcast([P, P]),
                            in1=iota_t[:],
                            op=mybir.AluOpType.is_equal)
                        is_first = done_of[h][k] == 0
                        is_last_pair = done_of[h][k] == npairs_of[h][k] - 1
                        last_phase = (phases_done[k] == phases_of[k] - 1)
                        nc.tensor.matmul(out=pacc_ap, lhsT=S[:],
                                         rhs=he[:, j * C:(j + 1) * C],
                                         start=is_first,
                                         stop=(is_last_pair and not last_phase))
                        done_of[h][k] += 1
                        if done_of[h][k] == npairs_of[h][k]:
                            bank = bank_of.pop(k)
                            phases_done[k] += 1
                            if phases_done[k] < phases_of[k]:
                                # spill partial into sbuf accumulator
                                if phases_done[k] == 1:
                                    nc.scalar.activation(
                                        out=acc[k][:], in_=bank[:],
                                        func=mybir.ActivationFunctionType.Copy)
                                else:
                                    nc.vector.tensor_add(
                                        out=acc[k][:], in0=acc[k][:],
                                        in1=bank[:])
                            else:
                                # last phase: fold self-loop + bias into psum
                                nc.tensor.matmul(
                                    out=bank[:], lhsT=identsl[:],
                                    rhs=hres[k][:], start=False, stop=False)
                                dv = dvpool.tile([1, P], bf16, tag="dv",
                                                 name=f"dv_{C}_{k}")
                                nc.sync.dma_start(
                                    out=dv[:], in_=dinvr[:, k * P:(k + 1) * P])
                                nc.tensor.matmul(
                                    out=bank[:], lhsT=dv[:],
                                    rhs=btile[:], start=False, stop=True)
                                epilogue(k, bank[:])

                for hh in range(NTAB):
                    assert all(done_of[hh][k] == npairs_of[hh][k]
                               for k in npairs_of[hh])

            fired = set()

            def agg_next(k):
                for t in range(NTAB):
                    if k == SLOT0[t] + SLOTS[t] - 1 and t != NTAB - 1:
                        fired.add(t)
                        nc.gpsimd.collective_compute(
                            "AllGather", mybir.AluOpType.bypass,
                            replica_groups=[list(range(CORES))],
                            ins=[ag2[t].opt()], outs=[tab2[t].opt()])

            def defer_last_ag2():
                nc.gpsimd.collective_compute(
                    "AllGather", mybir.AluOpType.bypass,
                    replica_groups=[list(range(CORES))],
                    ins=[ag2[NTAB - 1].opt()], outs=[tab2[NTAB - 1].opt()])

            def mk_ag1(t):
                def fire():
                    nc.gpsimd.collective_compute(
                        "AllGather", mybir.AluOpType.bypass,
                        replica_groups=[list(range(CORES))],
                        ins=[ag1[t].opt()], outs=[tab1[t].opt()])
                return fire

            conv(tab1, ag1, C1, he1pool, "he1", bf16, bf16, b1t,
                 last=False, agg_next=agg_next,
                 defers={1: mk_ag1(1), 2: mk_ag1(2)})
            conv(tab2, ag2, C2, he2pool, "he2", f32, f32, b2t,
                 last=True, defers={5: defer_last_ag2})

    nc.compile()
    return nc


_cache = {}


def kernel(x, edge_index, emb_a, emb_b, W1, b1, W2, b2):
    in_maps, meta = prep(x, edge_index, emb_a, emb_b, W1, b1, W2, b2)
    key = (meta["GCOLS"], meta["NPAIRS"], meta["MAXCH"],
           tuple((op[0], op[1], op[2], op[3], op[4], tuple(op[5]))
                 for op in meta["ops_seq"]))
    if key not in _cache:
        _cache[key] = build(meta)
    nc = _cache[key]
    res = run_bass_kernel_spmd(nc, in_maps, core_ids=list(range(CORES)))
    out = np.zeros((N, C2), dtype=np.float32)
    for c in range(CORES):
        yc = res.results[c]["y"]
        nodes = np.concatenate(
            [t * P + np.arange(P) for t in meta["core_tiles"][c]])
        valid = nodes < N
        out[nodes[valid]] = yc[valid]
    return out
